# revision 40
# baseline (speedup 1.0000x reference)
"""BoundaryLoss Trainium2 kernel.

Computes mean((pred_boundary*w - target_boundary*w)^2) where boundaries are
|conv3d(x, sobel)| of argmax-class / target volumes, w = 3 where target in
SMALL_CLASSES else 1.

Sharding: data-parallel over 8 cores = 2 batches x 4 depth-chunks of 32
slices (+1 halo slice each side, host zero-padded). Each core returns
per-(partition, group) partial sums of 576*(pb-tb)^2*w^2; host does the
mean (the scalar all-reduce across shards).

Math used on-device (exact in fp16/f32 integer arithmetic):
  conv3d(x, K) = (32*x - S_d S_h S_w x) / 24,  S = [1,2,1] separable
  A := 32*pred - S(pred) = 24*pred_boundary_signed   (pred scaled by 32 on dev)
  B := 32*t - S(t) = 24*targ_boundary_signed         (t kept 1x; PSUM holds B/32)
  loss partial = sum( ((|A| - |B|) * w)^2 ) = 576 * sum((pb-tb)^2 w^2)

Argmax trick: key_c = int32bits(v_c + 100.0) << 4 | (15 - c). v+100 in
[90,110) has fixed f32 exponent, so int key order == float order of v, and
low 4 bits give exact first-index tie-breaking. Max tree over 11 keys, then
idx = 15 - (key & 15), pred*32 = 480 - 32*(key & 15).
"""

import numpy as np

B, C, D, H, W = 2, 11, 128, 128, 128
N_CORES = 8
DSH = 32            # output depth slices per shard
DH = DSH + 2        # input slices incl. halo
CHUNKS = (8, 8, 8, 8, 2)
N_GROUPS = DSH // 4  # 8 PSUM groups of 4 output slices

_CACHE = {}


def _group_schedule(chunks, n_groups):
    """groups emitted after each chunk: group g needs input slices <= 4g+5."""
    sched, done = [], 0
    end = 0
    for nd in chunks:
        end += nd
        gs = []
        while done < n_groups and 4 * done + 5 <= end - 1:
            gs.append(done)
            done += 1
        sched.append(gs)
    assert done == n_groups, (sched, done)
    return sched


def _make_wmats():
    """[3,128,128] fp16: identity, -T/32, -T/16 with T = tridiag(1,2,1)."""
    T = np.zeros((128, 128), np.float32)
    i = np.arange(128)
    T[i, i] = 2.0
    T[i[:-1], i[:-1] + 1] = 1.0
    T[i[:-1] + 1, i[:-1]] = 1.0
    wm = np.stack([np.eye(128, dtype=np.float32), -T / 32.0, -T / 16.0])
    return wm.astype(np.float16)


def _build_nc(dh, chunks, debug=False, reps=1):
    import concourse.bass as bass  # noqa: F401
    import concourse.bacc as bacc
    import concourse.mybir as mybir
    from concourse.tile import TileContext

    f32, f16, i32 = mybir.dt.float32, mybir.dt.float16, mybir.dt.int32
    A = mybir.AluOpType
    AF = mybir.ActivationFunctionType
    X = mybir.AxisListType.X  # noqa: F841

    dsh = dh - 2
    n_groups = dsh // 4
    sched = _group_schedule(chunks, n_groups)
    max_nd = max(chunks)

    nc = bacc.Bacc()
    lg = nc.declare_dram_parameter("logits", [C, dh, H, W], f32, isOutput=False)
    tg = nc.declare_dram_parameter("target", [dh, H, W], i32, isOutput=False)
    wm = nc.declare_dram_parameter("wmats", [3, 128, 128], f16, isOutput=False)
    out = nc.declare_dram_parameter("out", [128, n_groups], f32, isOutput=True)
    if debug:
        dbg_pred = nc.declare_dram_parameter("dbg_pred", [128, dh, 130], f32, isOutput=True)
        dbg_wmap = nc.declare_dram_parameter("dbg_wmap", [128, dh, 128], f32, isOutput=True)
        dbg_xswp = nc.declare_dram_parameter("dbg_xswp", [128, dh, 128], f32, isOutput=True)
        dbg_psa = nc.declare_dram_parameter("dbg_psa", [128, n_groups, 512], f32, isOutput=True)
        dbg_psb = nc.declare_dram_parameter("dbg_psb", [128, n_groups, 512], f32, isOutput=True)

    PW = 130  # width padded with a zero column each side

    with TileContext(nc) as tc:
        from contextlib import ExitStack

        with ExitStack() as ctx:
            cpool = ctx.enter_context(tc.tile_pool(name="const", bufs=1))
            lgpool = ctx.enter_context(tc.tile_pool(name="lg", bufs=2 * C))
            tgpool = ctx.enter_context(tc.tile_pool(name="tgt", bufs=2))
            pers = ctx.enter_context(tc.tile_pool(name="pers", bufs=1))
            wkpool = ctx.enter_context(tc.tile_pool(name="wk", bufs=3))
            uvpool = ctx.enter_context(tc.tile_pool(name="uv", bufs=4))
            pspool = ctx.enter_context(tc.tile_pool(name="ps", bufs=4, space="PSUM"))

            # constants
            wt = cpool.tile([128, 3, 128], f16, tag="wt")
            nc.sync.dma_start(out=wt[:, :, :], in_=wm[:, :, :].rearrange("k h m -> h k m"))
            W_I, W_T1, W_T2 = wt[:, 0, :], wt[:, 1, :], wt[:, 2, :]

            # persistent volumes (halo-resident in SBUF)
            ptP = pers.tile([128, dh, PW], f16, tag="ptP")   # 32*pred, w-padded
            ptT = pers.tile([128, dh, PW], f16, tag="ptT")   # target,  w-padded
            xswP = pers.tile([128, dh, 128], f16, tag="xswP")
            xswT = pers.tile([128, dh, 128], f16, tag="xswT")
            wmap = pers.tile([128, dh, 128], f16, tag="wmap")
            acc = pers.tile([128, n_groups], f32, tag="acc")

            # zero whole padded buffers once (on ACT so POOL readers of ptT
            # only ever see ACT history); interior is overwritten per chunk
            nc.scalar.memzero(ptP[:, :, :])
            nc.scalar.memzero(ptT[:, :, :])

            def emit_group(g):
                psA = pspool.tile([128, 512], f32, tag="ps")
                psB = pspool.tile([128, 512], f32, tag="ps")
                # identity taps: +32p / +t
                nc.tensor.matmul(psA[:, :], W_I, ptP[:, 4 * g + 1 : 4 * g + 5, 1:129],
                                 start=True, stop=False)
                nc.tensor.matmul(psB[:, :], W_I, ptT[:, 4 * g + 1 : 4 * g + 5, 1:129],
                                 start=True, stop=False)
                # d-1 and d+1 taps (-T/32)
                nc.tensor.matmul(psA[:, :], W_T1, xswP[:, 4 * g : 4 * g + 4, :],
                                 start=False, stop=False)
                nc.tensor.matmul(psA[:, :], W_T1, xswP[:, 4 * g + 2 : 4 * g + 6, :],
                                 start=False, stop=False)
                nc.tensor.matmul(psB[:, :], W_T1, xswT[:, 4 * g : 4 * g + 4, :],
                                 start=False, stop=False)
                nc.tensor.matmul(psB[:, :], W_T1, xswT[:, 4 * g + 2 : 4 * g + 6, :],
                                 start=False, stop=False)
                # d tap (-T/16)
                nc.tensor.matmul(psA[:, :], W_T2, xswP[:, 4 * g + 1 : 4 * g + 5, :],
                                 start=False, stop=True)
                nc.tensor.matmul(psB[:, :], W_T2, xswT[:, 4 * g + 1 : 4 * g + 5, :],
                                 start=False, stop=True)
                # |A|, |B| (B accumulated at 1/32 scale -> scale=32 on abs)
                u = uvpool.tile([128, 512], f16, tag="u")
                v = uvpool.tile([128, 512], f16, tag="v")
                if debug:
                    du = uvpool.tile([128, 512], f32, tag="du")
                    dv = uvpool.tile([128, 512], f32, tag="dv")
                    nc.vector.tensor_copy(du[:, :], psA[:, :])
                    nc.vector.tensor_copy(dv[:, :], psB[:, :])
                    nc.sync.dma_start(
                        out=dbg_psa[:, g, :].rearrange("p w -> p w"), in_=du[:, :])
                    nc.sync.dma_start(
                        out=dbg_psb[:, g, :].rearrange("p w -> p w"), in_=dv[:, :])
                nc.scalar.activation(u[:, :], psA[:, :], AF.Abs)
                nc.scalar.activation(v[:, :], psB[:, :], AF.Abs, scale=32.0)
                e = wkpool.tile([128, 512], f16, tag="e")
                ew = wkpool.tile([128, 512], f16, tag="ew")
                scr = wkpool.tile([128, 512], f32, tag="scr")
                nc.gpsimd.tensor_tensor(e[:, :], u[:, :], v[:, :], A.subtract)
                nc.gpsimd.tensor_tensor(ew[:, :], e[:, :],
                                        wmap[:, 4 * g + 1 : 4 * g + 5, :], A.mult)
                nc.scalar.activation(scr[:, :], ew[:, :], AF.Square,
                                     accum_out=acc[:, g : g + 1])

            # optional on-device repeat loop (timing harness only; the acc
            # columns are overwritten, not accumulated, so reps are idempotent)
            rep_cm = tc.For_i(0, reps, 1) if reps > 1 else None
            if rep_cm is not None:
                rep_cm.__enter__()
            d0 = 0
            for ci, nd in enumerate(chunks):
                FD = nd * 128
                # --- DMA logits chunk (11 classes) + target chunk ---
                lts = []
                for c in range(C):
                    t = lgpool.tile([128, max_nd, 128], f32, tag="lg")
                    nc.sync.dma_start(
                        out=t[:, 0:nd, :],
                        in_=lg[c, d0 : d0 + nd, :, :].rearrange("d h w -> h d w"),
                    )
                    lts.append(t)
                tgt = tgpool.tile([128, max_nd, 128], i32, tag="tgt")
                nc.sync.dma_start(
                    out=tgt[:, 0:nd, :],
                    in_=tg[d0 : d0 + nd, :, :].rearrange("d h w -> h d w"),
                )

                def F(t):  # flat f32 view [128, FD]
                    return t[:, 0:nd, :].rearrange("p d w -> p (d w)")

                def I(t):  # flat int32 view
                    return F(t).bitcast(i32)

                # --- argmax keys ---
                # POOL instructions cannot wait on DMAHW semaphores (walrus
                # codegen limit), so POOL only ever reads ACT/POOL-written
                # tiles: classes 6-10 keys go to POOL-private kt tiles.
                # k1 = v + 100.0 (ACT, in-place)
                for c in range(C):
                    nc.scalar.activation(F(lts[c]), F(lts[c]), AF.Copy, bias=100.0)
                # k2 = (bits & ~15) | (15 - c) (DVE only: POOL lacks bitwise)
                for c in range(C):
                    nc.vector.tensor_scalar(I(lts[c]), I(lts[c]), -16, 15 - c,
                                            A.bitwise_and, A.bitwise_or)
                # max tree, all on DVE (POOL has no TT-max opcode)
                mx = nc.vector.tensor_tensor
                mx(F(lts[0]), F(lts[0]), F(lts[1]), A.max)
                mx(F(lts[2]), F(lts[2]), F(lts[3]), A.max)
                mx(F(lts[4]), F(lts[4]), F(lts[5]), A.max)
                mx(F(lts[6]), F(lts[6]), F(lts[7]), A.max)
                mx(F(lts[8]), F(lts[8]), F(lts[9]), A.max)
                mx(F(lts[0]), F(lts[0]), F(lts[2]), A.max)
                mx(F(lts[4]), F(lts[4]), F(lts[6]), A.max)
                mx(F(lts[8]), F(lts[8]), F(lts[10]), A.max)
                mx(F(lts[0]), F(lts[0]), F(lts[4]), A.max)
                mx(F(lts[0]), F(lts[0]), F(lts[8]), A.max)
                # extract: jt = key & 15; pred*32 = 480 - 32*jt
                nc.vector.tensor_scalar(I(lts[1]), I(lts[0]), 15, None,
                                        A.bitwise_and)
                nc.scalar.activation(ptP[:, d0 : d0 + nd, 1:129],
                                     lts[1][:, 0:nd, :].bitcast(i32),
                                     AF.Copy, scale=-32.0, bias=480.0)
                # target cast int32 -> f16
                nc.scalar.activation(ptT[:, d0 : d0 + nd, 1:129],
                                     tgt[:, 0:nd, :], AF.Copy)

                # --- weight map: w = 1 + [t<2] * 2 + [t==4] * 2 ... wait
                # small classes {2,3,5,..,10} get 3; {0,1,4} get 1:
                # w = 3 - 2*([t<2] + [t==4]) -> a=(t<2)*-2, b=(t==4)*-2, w=a+3+b
                wa = wkpool.tile([128, max_nd, 128], f16, tag="wa")
                wb = wkpool.tile([128, max_nd, 128], f16, tag="wb")
                wa_f = wa[:, 0:nd, :].rearrange("p d w -> p (d w)")
                wb_f = wb[:, 0:nd, :].rearrange("p d w -> p (d w)")
                tg16 = ptT[:, d0 : d0 + nd, 1:129]
                nc.gpsimd.tensor_scalar(wa_f, tg16, 2.0, -2.0, A.is_lt, A.mult)
                nc.gpsimd.tensor_scalar(wb_f, tg16, 4.0, -2.0, A.is_equal, A.mult)
                nc.vector.scalar_tensor_tensor(
                    wmap[:, d0 : d0 + nd, :], wa_f, 3.0, wb_f, A.add, A.add)

                # --- S_w: x = p[w-1] + 2 p[w] + p[w+1] (pred: DVE, tgt: POOL) ---
                nc.vector.scalar_tensor_tensor(
                    xswP[:, d0 : d0 + nd, :], ptP[:, d0 : d0 + nd, 1:129], 2.0,
                    ptP[:, d0 : d0 + nd, 0:128], A.mult, A.add)
                nc.gpsimd.tensor_tensor(
                    xswP[:, d0 : d0 + nd, :], xswP[:, d0 : d0 + nd, :],
                    ptP[:, d0 : d0 + nd, 2:130], A.add)
                nc.gpsimd.tensor_scalar(
                    xswT[:, d0 : d0 + nd, :], ptT[:, d0 : d0 + nd, 1:129],
                    2.0, None, A.mult)
                nc.gpsimd.tensor_tensor(
                    xswT[:, d0 : d0 + nd, :], xswT[:, d0 : d0 + nd, :],
                    ptT[:, d0 : d0 + nd, 0:128], A.add)
                nc.gpsimd.tensor_tensor(
                    xswT[:, d0 : d0 + nd, :], xswT[:, d0 : d0 + nd, :],
                    ptT[:, d0 : d0 + nd, 2:130], A.add)

                for g in sched[ci]:
                    emit_group(g)
                d0 += nd
            if rep_cm is not None:
                rep_cm.__exit__(None, None, None)

            if debug:
                dp = pers.tile([128, dh, 130], f32, tag="dp")
                dw = pers.tile([128, dh, 128], f32, tag="dw")
                dx = pers.tile([128, dh, 128], f32, tag="dx")
                nc.vector.tensor_copy(dp[:, :, :], ptP[:, :, :])
                nc.vector.tensor_copy(dw[:, :, :], wmap[:, :, :])
                nc.vector.tensor_copy(dx[:, :, :], xswP[:, :, :])
                nc.sync.dma_start(out=dbg_pred[:, :, :], in_=dp[:, :, :])
                nc.sync.dma_start(out=dbg_wmap[:, :, :], in_=dw[:, :, :])
                nc.sync.dma_start(out=dbg_xswp[:, :, :], in_=dx[:, :, :])
            nc.sync.dma_start(out=out[:, :], in_=acc[:, :])
    nc.compile()
    return nc


def _get_built(dh=DH, chunks=CHUNKS):
    key = (dh, tuple(chunks))
    if key not in _CACHE:
        _CACHE[key] = _build_nc(dh, chunks)
    return _CACHE[key]


def _shard_inputs(logits, target):
    """FULL inputs -> list of 8 per-core in_maps (b-major, then depth chunk)."""
    lp = np.zeros((B, C, D + 2, H, W), np.float32)
    lp[:, :, 1:-1] = logits
    tp = np.zeros((B, 1, D + 2, H, W), np.int32)
    tp[:, :, 1:-1] = target
    wm = _make_wmats()
    maps = []
    for b in range(B):
        for j in range(D // DSH):
            s = j * DSH
            maps.append({
                "logits": np.ascontiguousarray(lp[b, :, s : s + DH]),
                "target": np.ascontiguousarray(tp[b, 0, s : s + DH]),
                "wmats": wm,
            })
    return maps


def kernel(logits: np.ndarray, target: np.ndarray) -> np.ndarray:
    from concourse.bass_utils import run_bass_kernel_spmd

    nc = _get_built()
    maps = _shard_inputs(np.asarray(logits), np.asarray(target))
    res = run_bass_kernel_spmd(nc, maps, list(range(N_CORES))).results
    total = 0.0
    for r in res:
        total += np.asarray(r["out"], np.float64).sum()
    loss = total / (576.0 * B * D * H * W)
    return np.float32(loss)


# ---------------- numpy reference for one shard (testing only) ----------------

def shard_partial_np(lg, tgt):
    """lg [C,dh,H,W] f32 (already +halo, zero-padded), tgt [dh,H,W] i32.
    Returns sum over interior slices of 576*(pb-tb)^2*w^2."""
    pred = np.argmax(lg, axis=0).astype(np.float32)
    t = tgt.astype(np.float32)

    def S(x):
        xp = np.pad(x, ((0, 0), (1, 1), (1, 1)))
        s = xp[:, :, :-2] + 2 * xp[:, :, 1:-1] + xp[:, :, 2:]
        s = s[:, :-2, :] + 2 * s[:, 1:-1, :] + s[:, 2:, :]
        return s[:-2] + 2 * s[1:-1] + s[2:]

    Av = 32 * pred[1:-1] - S(pred)
    Bv = 32 * t[1:-1] - S(t)
    w = np.where((tgt[1:-1] < 2) | (tgt[1:-1] == 4), 1.0, 3.0).astype(np.float32)
    e = (np.abs(Av) - np.abs(Bv)) * w
    return float(np.sum((e * e).astype(np.float64)))
